# revision 7
# baseline (speedup 1.0000x reference)
"""MHA layer (QKV proj + masked softmax attention + out proj + residual + LayerNorm)
on 8 NeuronCores. Sharding: batch(4) x query-set(2). No collectives: each core
computes K/V for its full batch, Q only for its assigned 1024 rows.

Key optimizations over the naive layout:
- Host ships x^T and W pre-quantized to fp8e4 in DoubleRow [64,2,*] layout:
  Q/K/V projections run as fp8 DoubleRow matmuls (0.5 cyc/col), no DMA
  transpose needed on-chip.
- Query compaction: the mask zeroes whole query rows (their attention output
  is mean(V)); host permutes each core's 1024 rows so <=639 unmasked rows
  land in a 640-slot compact block that flows through scores/exp/attV/
  out-proj; the remaining 384 masked rows take a cheap shared
  mean(V)@Wp + bp path. Host inverse-permutes the output.
- Scores/attV operands (qt/kt/vaug/ex) in fp8 to cut SBUF + DVE traffic.

Self-contained: hardcodes shapes from the problem spec.
"""

import os
import numpy as np

import concourse.bass as bass
import concourse.bacc as bacc
import concourse.tile as tile
import concourse.mybir as mybir
from concourse.bass_utils import run_bass_kernel_spmd

B, T, C, H, D = 4, 2048, 1024, 16, 64
TQ = T // 2          # query rows per core
TQC = 640            # compact attention slots (incl. masked fill; slot 639 masked)
NQB = TQC // 128     # compact row tiles (5)
N_CORES = 8
P = 128
NJ = C // P          # 8 c-chunks
NTK = T // P         # 16 key chunks
QB = 320             # query block for scores/exp/attV (psum half-bank)
NQBLK = TQC // QB    # 2
LN_EPS = 1e-5
VSLOT = 66           # V_aug per-head slot: 64 V cols + 1 ones + 1 pad

f32 = mybir.dt.float32
bf16 = mybir.dt.bfloat16
f8 = mybir.dt.float8e4
AX = mybir.AxisListType
ALU = mybir.AluOpType
ACTF = mybir.ActivationFunctionType
DR = mybir.MatmulPerfMode.DoubleRow


def build(affine: bool):
    phase_lim = int(os.environ.get("K_PHASE", "4"))
    repeat = int(os.environ.get("K_REPEAT", "1"))
    nc = bacc.Bacc("TRN2", target_bir_lowering=False, debug=False,
                   num_devices=N_CORES)

    # x^T in fp8 DoubleRow layout: row i*64+p, col two*T+t = x[t, i*128+two*64+p]
    xt8 = nc.dram_tensor("xt8", [NJ * 64, 2 * T], f8, kind="ExternalInput")
    # compacted query rows, same layout over TQC columns
    xq8 = nc.dram_tensor("xq8", [NJ * 64, 2 * TQC], f8, kind="ExternalInput")
    # Wq,Wk,Wv (scaled) stacked in DoubleRow layout
    w8 = nc.dram_tensor("w8", [3 * NJ * 64, 2 * C], f8, kind="ExternalInput")
    wpb = nc.dram_tensor("wpb", [C, C], bf16, kind="ExternalInput")
    # fx rows: 0..TQ-1 xres (permuted); TQ+0 bq*; +1 bk*; +2 bv; +3 bp;
    # +4 lng; +5 lnb; +6 maskq (scaled, cols 0:TQC)
    fx = nc.dram_tensor("fx", [TQ + 7, C], f32, kind="ExternalInput")
    outd = nc.dram_tensor("out", [TQ, C], f32, kind="ExternalOutput")

    with tile.TileContext(nc) as tc:
        with (
            tc.tile_pool(name="pers", bufs=1) as pers,
            tc.tile_pool(name="ev", bufs=2) as evp,
            tc.tile_pool(name="ex", bufs=6) as exp_pool,
            tc.tile_pool(name="sm", bufs=2) as smp,
            tc.tile_pool(name="psum", bufs=1, space=bass.MemorySpace.PSUM) as psp,
        ):
            def _iter_body():
                # ---- phase A: small loads, broadcasts, big fp8 loads ----
                mrow_f = smp.tile([1, TQC], f32, tag="mrowf", bufs=1,
                                  name="mrow_f")
                nc.sync.dma_start(mrow_f[:], fx[TQ + 6:TQ + 7, 0:TQC])
                mrow = pers.tile([1, TQC], bf16, tag="mrow")
                nc.vector.tensor_copy(mrow[:], mrow_f[:])
                bvrow_f = smp.tile([1, C], f32, tag="rowf", bufs=1,
                                   name="bvrow_f")
                nc.sync.dma_start(bvrow_f[:], fx[TQ + 2:TQ + 3, :])
                bvrow = smp.tile([1, C], bf16, tag="rowb", bufs=1,
                                 name="bvrow")
                nc.vector.tensor_copy(bvrow[:], bvrow_f[:])
                bv_bc = pers.tile([P, C], bf16, tag="bv_bc")
                nc.gpsimd.partition_broadcast(bv_bc[:], bvrow[:])
                bq_t = pers.tile([P, NJ], f32, tag="bq_t")
                nc.sync.dma_start(bq_t[:],
                                  fx[TQ + 0:TQ + 1, :].rearrange(
                                      "a (j p) -> p (a j)", p=P))
                bk_t = pers.tile([P, NJ], f32, tag="bk_t")
                nc.sync.dma_start(bk_t[:],
                                  fx[TQ + 1:TQ + 2, :].rearrange(
                                      "a (j p) -> p (a j)", p=P))

                eps_t = pers.tile([P, 1], f32, tag="eps_t")
                nc.gpsimd.memset(eps_t[:], LN_EPS)
                mask_bc = pers.tile([P, TQC], bf16, tag="mask_bc")
                nc.gpsimd.partition_broadcast(mask_bc[:], mrow[:])
                bprow_f = smp.tile([1, C], f32, tag="rowf", bufs=1,
                                   name="bprow_f")
                nc.sync.dma_start(bprow_f[:], fx[TQ + 3:TQ + 4, :])
                bprow_b = smp.tile([1, C], bf16, tag="rowb", bufs=1,
                                   name="bprow_b")
                nc.vector.tensor_copy(bprow_b[:], bprow_f[:])
                bp_bc = pers.tile([P, C], bf16, tag="bp_bc")
                nc.gpsimd.partition_broadcast(bp_bc[:], bprow_b[:])
                if affine:
                    lngrow = pers.tile([1, C], f32, tag="lngrow")
                    nc.sync.dma_start(lngrow[:], fx[TQ + 4:TQ + 5, :])
                    lnbrow = pers.tile([1, C], f32, tag="lnbrow")
                    nc.sync.dma_start(lnbrow[:], fx[TQ + 5:TQ + 6, :])
                    lng_bc = pers.tile([P, C], f32, tag="lng_bc")
                    nc.gpsimd.partition_broadcast(lng_bc[:], lngrow[:])
                    lnb_bc = pers.tile([P, C], f32, tag="lnb_bc")
                    nc.gpsimd.partition_broadcast(lnb_bc[:], lnbrow[:])

                # big fp8 operand loads (already in DoubleRow layout in DRAM)
                xtd, xqd, wqd, wkd, wvd = [], [], [], [], []
                for i in range(NJ):
                    t_ = pers.tile([64, 2 * T], f8, tag=f"xtd{i}", name=f"xtd{i}")
                    nc.sync.dma_start(t_[:], xt8[i * 64:(i + 1) * 64, :])
                    xtd.append(t_)
                    t_ = pers.tile([64, 2 * TQC], f8, tag=f"xqd{i}", name=f"xqd{i}")
                    nc.sync.dma_start(t_[:], xq8[i * 64:(i + 1) * 64, :])
                    xqd.append(t_)
                    for m, slot, lst in ((2, i, wvd), (1, NJ + i, wkd)):
                        t_ = pers.tile([64, 2 * C], f8, tag=f"w8s_{slot}",
                                       name=f"w8s_{slot}")
                        nc.sync.dma_start(
                            t_[:], w8[(m * NJ + i) * 64:(m * NJ + i + 1) * 64, :])
                        lst.append(t_)
                wpd = []
                for j in range(NJ):
                    t_ = pers.tile([P, C], bf16, tag=f"wpd{j}", name=f"wpd{j}")
                    nc.sync.dma_start(t_[:], wpb[j * P:(j + 1) * P, :])
                    wpd.append(t_)

                def r3t(t_):   # [64, 2*N] tile -> [64, 2, N] AP
                    return t_[:].rearrange("p (two n) -> p two n", two=2)

                # ---- persistent attention operands ----
                qt = [pers.tile([P, TQC], f8, tag=f"qt{j}", name=f"qt{j}")
                      for j in range(NJ)]
                kt = [pers.tile([P, T], f8, tag=f"kt{j}", name=f"kt{j}")
                      for j in range(NJ)]
                vaug = [pers.tile([P, H * VSLOT], f8, tag=f"va{t}", name=f"va{t}")
                        for t in range(NTK)]
                yt = [pers.tile([P, TQC], bf16, tag=f"yt{j}", name=f"yt{j}")
                      for j in range(NJ)]

                # ---- phase B1: V = x @ Wv + bv -> vaug (+ ones col) ----
                if phase_lim >= 1:
                    for tk in range(NTK):
                        ones_ap = vaug[tk][:].rearrange("p (h e) -> p h e",
                                                        e=VSLOT)
                        nc.gpsimd.memset(ones_ap[:, :, 64:65], 1.0)
                    for d2 in range(2):
                        for tk in range(NTK):
                            psv = psp.tile([P, 512], f32, tag="sc", bufs=4, name=f"psv{d2}_{tk}")
                            for i in range(NJ):
                                nc.tensor.matmul(
                                    psv[:],
                                    r3t(xtd[i])[:, :, tk * P:(tk + 1) * P],
                                    r3t(wvd[i])[:, :, d2 * 512:(d2 + 1) * 512],
                                    start=(i == 0), stop=(i == NJ - 1),
                                    perf_mode=DR)
                            dst = vaug[tk][:].rearrange("p (h e) -> p h e",
                                                        e=VSLOT)
                            nc.vector.scalar_tensor_tensor(
                                dst[:, 8 * d2:8 * d2 + 8, 0:64],
                                psv[:].rearrange("p (h d) -> p h d", d=D),
                                INV_SV,
                                bv_bc[:, d2 * 512:(d2 + 1) * 512].rearrange(
                                    "p (h d) -> p h d", d=D),
                                op0=ALU.mult, op1=ALU.add)

                # wq reuses the wv tile slots (V proj done by then)
                if phase_lim >= 2:
                    for i in range(NJ):
                        t_ = pers.tile([64, 2 * C], f8, tag=f"w8s_{i}",
                                       name=f"wq_{i}")
                        nc.sync.dma_start(
                            t_[:], w8[(0 * NJ + i) * 64:(0 * NJ + i + 1) * 64, :])
                        wqd.append(t_)

                # ---- phase B2: per c-chunk j: Q^T, K^T ----
                def qk_produce(j):
                    for blk in range(NQBLK):
                        psq = psp.tile([P, 512], f32, tag="sc", bufs=4,
                                       name=f"psq{j}_{blk}")
                        for i in range(NJ):
                            nc.tensor.matmul(
                                psq[:, 0:QB],
                                r3t(wqd[i])[:, :, j * P:(j + 1) * P],
                                r3t(xqd[i])[:, :, blk * QB:(blk + 1) * QB],
                                start=(i == 0), stop=(i == NJ - 1),
                                perf_mode=DR)
                        # qt = (psq + bq*) * mask*  (masked rows -> 0 scores)
                        nc.vector.scalar_tensor_tensor(
                            qt[j][:, blk * QB:(blk + 1) * QB], psq[:, 0:QB],
                            bq_t[:, j:j + 1],
                            mask_bc[:, blk * QB:(blk + 1) * QB],
                            op0=ALU.add, op1=ALU.mult)
                    for kb in range(T // 512):
                        psk = psp.tile([P, 512], f32, tag="sc", bufs=4,
                                       name=f"psk{j}_{kb}")
                        for i in range(NJ):
                            nc.tensor.matmul(
                                psk[:],
                                r3t(wkd[i])[:, :, j * P:(j + 1) * P],
                                r3t(xtd[i])[:, :, kb * 512:(kb + 1) * 512],
                                start=(i == 0), stop=(i == NJ - 1),
                                perf_mode=DR)
                        nc.vector.tensor_scalar(
                            kt[j][:, kb * 512:(kb + 1) * 512], psk[:],
                            INV_SK, bk_t[:, j:j + 1],
                            op0=ALU.mult, op1=ALU.add)

                def attn_chunk(j):
                    yaccs = {}
                    for hh in range(2):
                        for blk in range(NQBLK):
                            yaccs[(hh, blk)] = psp.tile(
                                [65, 512], f32, tag="yacc", bufs=4,
                                name=f"yacc{j}_{hh}_{blk}")
                    for tk in range(NTK):
                        exs = {}
                        for hh in range(2):
                            pb = hh * 64
                            for blk in range(NQBLK):
                                pss = psp.tile([P, 512], f32, tag="sc", bufs=4,
                                               name=f"pss{j}_{hh}_{blk}")
                                nc.tensor.matmul(
                                    pss[:, 0:QB],
                                    kt[j][pb:pb + 64, tk * P:(tk + 1) * P],
                                    qt[j][pb:pb + 64,
                                          blk * QB:(blk + 1) * QB],
                                    start=True, stop=True,
                                    tile_position=(pb, 0))
                                ex = exp_pool.tile([P, QB], f8, tag="ex",
                                                   name=f"ex{j}_{hh}_{blk}")
                                nc.scalar.activation(ex[:], pss[:, 0:QB],
                                                     ACTF.Exp)
                                exs[(hh, blk)] = ex
                        for hh in range(2):
                            h = 2 * j + hh
                            for blk in range(NQBLK):
                                nc.tensor.matmul(
                                    yaccs[(hh, blk)][0:65, 0:QB],
                                    vaug[tk][:, h * VSLOT:h * VSLOT + 65],
                                    exs[(hh, blk)][:],
                                    start=(tk == 0), stop=(tk == NTK - 1))
                    for hh in range(2):
                        sr = smp.tile([P, TQC], f32, tag="sr")
                        for blk in range(NQBLK):
                            nc.vector.reciprocal(
                                sr[64:65, blk * QB:(blk + 1) * QB],
                                yaccs[(hh, blk)][64:65, 0:QB])
                        srb = smp.tile([1, TQC], f32, tag="srb", bufs=1)
                        nc.sync.dma_start(srb[:], sr[64:65, :])
                        nc.gpsimd.partition_broadcast(sr[0:64, :], srb[:])
                        if hh == 0:
                            for blk in range(NQBLK):
                                nc.vector.tensor_tensor(
                                    yt[j][0:64, blk * QB:(blk + 1) * QB],
                                    yaccs[(hh, blk)][0:64, 0:QB],
                                    sr[0:64, blk * QB:(blk + 1) * QB],
                                    op=ALU.mult)
                        else:
                            yo = smp.tile([64, TQC], bf16, tag="yo")
                            for blk in range(NQBLK):
                                nc.vector.tensor_tensor(
                                    yo[:, blk * QB:(blk + 1) * QB],
                                    yaccs[(hh, blk)][0:64, 0:QB],
                                    sr[0:64, blk * QB:(blk + 1) * QB],
                                    op=ALU.mult)
                            nc.sync.dma_start(yt[j][64:128, :], yo[:])

                if phase_lim >= 2:
                    if phase_lim < 3:
                        for j in range(NJ):
                            qk_produce(j)
                    else:
                        for j in range(NJ):
                            qk_produce(j)
                            attn_chunk(j)

                # ---- phase D: out proj + residual + LayerNorm ----
                if phase_lim >= 4:
                    # z = mean(V) @ Wp + bp from the meanV column (slot 639,
                    # guaranteed masked -> its normalized y == mean(V))
                    zc = pers.tile([1, C], f32, tag="zc")
                    for half in range(2):
                        psz = psp.tile([P, 512], f32, tag="sc", bufs=4,
                                       name=f"psz{half}")
                        for j in range(NJ):
                            nc.tensor.matmul(
                                psz[0:1, :],
                                yt[j][:, TQC - 1:TQC],
                                wpd[j][:, half * 512:(half + 1) * 512],
                                start=(j == 0), stop=(j == NJ - 1))
                        nc.vector.tensor_tensor(
                            zc[:, half * 512:(half + 1) * 512], psz[0:1, :],
                            bp_bc[0:1, half * 512:(half + 1) * 512],
                            op=ALU.add)
                    zbc = pers.tile([P, C], f32, tag="zbc")
                    nc.gpsimd.partition_broadcast(zbc[:], zc[:])

                    for i in range(TQ // P):
                        xr = evp.tile([P, C], f32, tag="xr", bufs=2,
                                      name=f"xr{i}")
                        nc.sync.dma_start(xr[:], fx[i * P:(i + 1) * P, :])
                        hres = evp.tile([P, C], f32, tag="hres", bufs=2)
                        if i < NQB:
                            for half in range(2):
                                pso = psp.tile([P, 512], f32, tag="sc", bufs=4,
                                               name=f"pso{i}_{half}")
                                for j in range(NJ):
                                    nc.tensor.matmul(
                                        pso[:],
                                        yt[j][:, i * P:(i + 1) * P],
                                        wpd[j][:, half * 512:(half + 1) * 512],
                                        start=(j == 0), stop=(j == NJ - 1))
                                nc.vector.tensor_tensor(
                                    hres[:, half * 512:(half + 1) * 512],
                                    pso[:],
                                    bp_bc[:, half * 512:(half + 1) * 512],
                                    op=ALU.add)
                            nc.vector.tensor_tensor(hres[:], hres[:], xr[:],
                                                    op=ALU.add)
                        else:
                            nc.vector.tensor_tensor(hres[:], zbc[:], xr[:],
                                                    op=ALU.add)
                        stat = smp.tile([P, 8], f32, tag="stat")
                        nc.vector.reduce_sum(stat[:, 0:1], hres[:], axis=AX.X)
                        sq = evp.tile([P, C], bf16, tag="sq", bufs=1)
                        nc.scalar.activation(sq[:], hres[:], ACTF.Square,
                                             accum_out=stat[:, 1:2])
                        nc.vector.tensor_scalar(stat[:, 2:3], stat[:, 0:1],
                                                1.0 / C, None, op0=ALU.mult)
                        nc.vector.tensor_scalar(stat[:, 3:4], stat[:, 1:2],
                                                1.0 / C, None, op0=ALU.mult)
                        nc.vector.tensor_tensor(stat[:, 4:5], stat[:, 2:3],
                                                stat[:, 2:3], op=ALU.mult)
                        nc.vector.tensor_tensor(stat[:, 5:6], stat[:, 3:4],
                                                stat[:, 4:5], op=ALU.subtract)
                        nc.scalar.activation(stat[:, 6:7], stat[:, 5:6],
                                             ACTF.Sqrt, bias=eps_t[:])
                        nc.vector.reciprocal(stat[:, 7:8], stat[:, 6:7])
                        nc.vector.tensor_scalar(hres[:], hres[:], stat[:, 2:3],
                                                stat[:, 7:8],
                                                op0=ALU.subtract, op1=ALU.mult)
                        if affine:
                            nc.vector.tensor_tensor(hres[:], hres[:],
                                                    lng_bc[:], op=ALU.mult)
                            nc.vector.tensor_tensor(hres[:], hres[:],
                                                    lnb_bc[:], op=ALU.add)
                        nc.sync.dma_start(outd[i * P:(i + 1) * P, :], hres[:])

            for _rep in range(repeat):
                _iter_body()

    nc.compile()
    return nc


# ---- host-side scales (set at import; recomputed per input in _make_in_maps) ----
INV_SV = 1.0
INV_SK = 1.0


_CACHE = {}


def _get_nc(affine: bool):
    if affine not in _CACHE:
        _CACHE[affine] = build(affine)
    return _CACHE[affine]


def _f8_layout(wT: np.ndarray, ncols: int, f8np) -> np.ndarray:
    """[C, ncols] f32 -> [NJ*64, 2*ncols] fp8 DoubleRow layout."""
    return (wT.reshape(NJ, 2, 64, ncols).transpose(0, 2, 1, 3)
            .reshape(NJ * 64, 2 * ncols).astype(f8np))


def _make_in_maps(x, Wq, bq, Wk, bk, Wv, bv, Wp, bp, ln_g, ln_b, mask,
                  affine: bool):
    global INV_SV, INV_SK
    f8np = mybir.dt.np(f8)
    bfnp = mybir.dt.np(bf16)
    x = np.asarray(x, np.float32)
    mask = np.asarray(mask)
    Wq = np.asarray(Wq, np.float32)
    Wk = np.asarray(Wk, np.float32)
    Wv = np.asarray(Wv, np.float32)
    Wp = np.asarray(Wp, np.float32)
    sc = np.float32(1.0 / np.sqrt(D))

    # fp8 scales: normalize each W to absmax ~224
    aQ = np.float32(224.0 / np.abs(Wq * sc).max())
    sK = np.float32(224.0 / np.abs(Wk).max())
    sV = np.float32(224.0 / np.abs(Wv).max())
    # these must match what build() baked in; build reads them at trace time,
    # so they are fixed constants of the compiled kernel -> bake via globals
    # before _get_nc() is called (see run()).
    INV_SV = float(1.0 / sV)
    INV_SK = float(1.0 / sK)

    w8_h = np.concatenate([
        _f8_layout(Wq * (sc * aQ), C, f8np),
        _f8_layout(Wk * sK, C, f8np),
        _f8_layout(Wv * sV, C, f8np)], axis=0)
    wpb_h = Wp.astype(bfnp)

    extra = np.stack([
        np.asarray(bq, np.float32) * sc * aQ,
        np.asarray(bk, np.float32),          # bk added after 1/sK rescale
        np.asarray(bv, np.float32),
        np.asarray(bp, np.float32),
        np.asarray(ln_g, np.float32), np.asarray(ln_b, np.float32),
        np.zeros(C, np.float32)], axis=0)

    in_maps = []
    perms = []
    for b in range(B):
        xb = x[b]
        xt8_h = _f8_layout(xb.T, T, f8np)
        mb = np.asarray(mask[b]) != 0
        U = np.nonzero(mb)[0]
        M = np.nonzero(~mb)[0]
        UA, UB = U[0::2], U[1::2]
        assert len(UA) <= TQC - 1 and len(UB) <= TQC - 1, (
            f"unmasked rows per core ({len(UA)}, {len(UB)}) exceed compact "
            f"capacity {TQC - 1}; mask too dense for this kernel build")
        nMA = TQ - len(UA)
        MA, MB = M[:nMA], M[nMA:]
        for half, (Uh, Mh) in enumerate(((UA, MA), (UB, MB))):
            perm = np.concatenate([Uh, Mh])
            perms.append(perm)
            fx_h = np.empty((TQ + 7, C), np.float32)
            fx_h[0:TQ] = xb[perm]
            fx_h[TQ:] = extra
            fx_h[TQ + 6, :] = 0.0
            fx_h[TQ + 6, 0:TQC] = mb[perm[:TQC]].astype(np.float32) / aQ
            xq8_h = _f8_layout(xb[perm[:TQC]].T, TQC, f8np)
            in_maps.append({
                "xt8": xt8_h, "xq8": xq8_h, "w8": w8_h, "wpb": wpb_h,
                "fx": fx_h,
            })
    return in_maps, perms


def run(inputs: dict, trace: bool = False):
    ln_g = np.asarray(inputs["ln_g"], np.float32)
    ln_b = np.asarray(inputs["ln_b"], np.float32)
    affine = not (np.all(ln_g == 1.0) and np.all(ln_b == 0.0))
    in_maps, perms = _make_in_maps(**inputs, affine=affine)
    nc = _get_nc(affine)
    res = None
    for attempt in range(3):
        try:
            res = run_bass_kernel_spmd(nc, in_maps, list(range(N_CORES)),
                                       trace=trace)
            break
        except Exception:
            if attempt == 2:
                raise
            import time as _time
            _time.sleep(2.0)
    out = np.empty((B, T, C), np.float32)
    for c in range(N_CORES):
        b = c // 2
        out[b, perms[c]] = res.results[c]["out"]
    return out, res


def kernel(**inputs) -> np.ndarray:
    out, _ = run(inputs, trace=False)
    return out


# revision 9
# speedup vs baseline: 1.0611x; 1.0611x over previous
"""MHA layer (QKV proj + masked softmax attention + out proj + residual + LayerNorm)
on 8 NeuronCores. Sharding: batch(4) x query-set(2). No collectives: each core
computes K/V for its full batch, Q only for its assigned 1024 rows.

Key optimizations over the naive layout:
- Host ships x^T and W pre-quantized to fp8e4 in DoubleRow [64,2,*] layout:
  Q/K/V projections run as fp8 DoubleRow matmuls (0.5 cyc/col), no DMA
  transpose needed on-chip.
- Query compaction: the mask zeroes whole query rows (their attention output
  is mean(V)); host permutes each core's 1024 rows so <=639 unmasked rows
  land in a 640-slot compact block that flows through scores/exp/attV/
  out-proj; the remaining 384 masked rows take a cheap shared
  mean(V)@Wp + bp path. Host inverse-permutes the output.
- Scores/attV operands (qt/kt/vaug/ex) in fp8 to cut SBUF + DVE traffic.

Self-contained: hardcodes shapes from the problem spec.
"""

import os
import numpy as np

import concourse.bass as bass
import concourse.bacc as bacc
import concourse.tile as tile
import concourse.mybir as mybir
from concourse.bass_utils import run_bass_kernel_spmd

B, T, C, H, D = 4, 2048, 1024, 16, 64
TQ = T // 2          # query rows per core
TQC = 640            # compact attention slots (incl. masked fill; slot 639 masked)
NQB = TQC // 128     # compact row tiles (5)
N_CORES = 8
P = 128
NJ = C // P          # 8 c-chunks
NTK = T // P         # 16 key chunks
QB = 320             # query block for scores/exp/attV (psum half-bank)
NQBLK = TQC // QB    # 2
LN_EPS = 1e-5
VSLOT = 66           # V_aug per-head slot: 64 V cols + 1 ones + 1 pad

f32 = mybir.dt.float32
bf16 = mybir.dt.bfloat16
f8 = mybir.dt.float8e4
AX = mybir.AxisListType
ALU = mybir.AluOpType
ACTF = mybir.ActivationFunctionType
DR = mybir.MatmulPerfMode.DoubleRow


def build(affine: bool):
    phase_lim = int(os.environ.get("K_PHASE", "4"))
    repeat = int(os.environ.get("K_REPEAT", "1"))
    nc = bacc.Bacc("TRN2", target_bir_lowering=False, debug=False,
                   num_devices=N_CORES)

    # x^T in fp8 DoubleRow layout: row i*64+p, col two*T+t = x[t, i*128+two*64+p]
    xt8 = nc.dram_tensor("xt8", [NJ * 64, 2 * T], f8, kind="ExternalInput")
    # compacted query rows, same layout over TQC columns
    xq8 = nc.dram_tensor("xq8", [NJ * 64, 2 * TQC], f8, kind="ExternalInput")
    # Wq,Wk,Wv (scaled) stacked in DoubleRow layout
    w8 = nc.dram_tensor("w8", [3 * NJ * 64, 2 * C], f8, kind="ExternalInput")
    wpb = nc.dram_tensor("wpb", [C, C], bf16, kind="ExternalInput")
    # fx rows: 0..TQ-1 xres (permuted); TQ+0 bq*; +1 bk*; +2 bv; +3 bp;
    # +4 lng; +5 lnb; +6 maskq (scaled, cols 0:TQC)
    fx = nc.dram_tensor("fx", [TQ + 7, C], f32, kind="ExternalInput")
    outd = nc.dram_tensor("out", [TQ, C], f32, kind="ExternalOutput")

    with tile.TileContext(nc) as tc:
        with (
            tc.tile_pool(name="pers", bufs=1) as pers,
            tc.tile_pool(name="ev", bufs=2) as evp,
            tc.tile_pool(name="ex", bufs=10) as exp_pool,
            tc.tile_pool(name="sm", bufs=2) as smp,
            tc.tile_pool(name="psum", bufs=1, space=bass.MemorySpace.PSUM) as psp,
        ):
            def _iter_body():
                # ---- phase A: small loads, broadcasts, big fp8 loads ----
                mrow_f = smp.tile([1, TQC], f32, tag="mrowf", bufs=1,
                                  name="mrow_f")
                nc.sync.dma_start(mrow_f[:], fx[TQ + 6:TQ + 7, 0:TQC])
                mrow = pers.tile([1, TQC], bf16, tag="mrow")
                nc.vector.tensor_copy(mrow[:], mrow_f[:])
                bvrow_f = smp.tile([1, C], f32, tag="rowf", bufs=1,
                                   name="bvrow_f")
                nc.sync.dma_start(bvrow_f[:], fx[TQ + 2:TQ + 3, :])
                bvrow = smp.tile([1, C], bf16, tag="rowb", bufs=1,
                                 name="bvrow")
                nc.vector.tensor_copy(bvrow[:], bvrow_f[:])
                bv_bc = pers.tile([P, C], bf16, tag="bv_bc")
                nc.gpsimd.partition_broadcast(bv_bc[:], bvrow[:])
                bq_t = pers.tile([P, NJ], f32, tag="bq_t")
                nc.sync.dma_start(bq_t[:],
                                  fx[TQ + 0:TQ + 1, :].rearrange(
                                      "a (j p) -> p (a j)", p=P))
                bk_t = pers.tile([P, NJ], f32, tag="bk_t")
                nc.sync.dma_start(bk_t[:],
                                  fx[TQ + 1:TQ + 2, :].rearrange(
                                      "a (j p) -> p (a j)", p=P))

                eps_t = pers.tile([P, 1], f32, tag="eps_t")
                nc.gpsimd.memset(eps_t[:], LN_EPS)
                mask_bc = pers.tile([P, TQC], bf16, tag="mask_bc")
                nc.gpsimd.partition_broadcast(mask_bc[:], mrow[:])
                bprow_f = smp.tile([1, C], f32, tag="rowf", bufs=1,
                                   name="bprow_f")
                nc.sync.dma_start(bprow_f[:], fx[TQ + 3:TQ + 4, :])
                bprow_b = smp.tile([1, C], bf16, tag="rowb", bufs=1,
                                   name="bprow_b")
                nc.vector.tensor_copy(bprow_b[:], bprow_f[:])
                bp_bc = pers.tile([P, C], bf16, tag="bp_bc")
                nc.gpsimd.partition_broadcast(bp_bc[:], bprow_b[:])
                if affine:
                    lngrow = pers.tile([1, C], f32, tag="lngrow")
                    nc.sync.dma_start(lngrow[:], fx[TQ + 4:TQ + 5, :])
                    lnbrow = pers.tile([1, C], f32, tag="lnbrow")
                    nc.sync.dma_start(lnbrow[:], fx[TQ + 5:TQ + 6, :])
                    lng_bc = pers.tile([P, C], f32, tag="lng_bc")
                    nc.gpsimd.partition_broadcast(lng_bc[:], lngrow[:])
                    lnb_bc = pers.tile([P, C], f32, tag="lnb_bc")
                    nc.gpsimd.partition_broadcast(lnb_bc[:], lnbrow[:])

                # big fp8 operand loads (already in DoubleRow layout in DRAM)
                xtd, xqd, wqd, wkd, wvd = [], [], [], [], []
                for i in range(NJ):
                    t_ = pers.tile([64, 2 * T], f8, tag=f"xtd{i}", name=f"xtd{i}")
                    nc.sync.dma_start(t_[:], xt8[i * 64:(i + 1) * 64, :])
                    xtd.append(t_)
                    t_ = pers.tile([64, 2 * TQC], f8, tag=f"xqd{i}", name=f"xqd{i}")
                    nc.sync.dma_start(t_[:], xq8[i * 64:(i + 1) * 64, :])
                    xqd.append(t_)
                    for m, slot, lst in ((2, i, wvd), (1, NJ + i, wkd)):
                        t_ = pers.tile([64, 2 * C], f8, tag=f"w8s_{slot}",
                                       name=f"w8s_{slot}")
                        nc.sync.dma_start(
                            t_[:], w8[(m * NJ + i) * 64:(m * NJ + i + 1) * 64, :])
                        lst.append(t_)
                wpd = []
                for j in range(NJ):
                    t_ = pers.tile([P, C], bf16, tag=f"wpd{j}", name=f"wpd{j}")
                    nc.sync.dma_start(t_[:], wpb[j * P:(j + 1) * P, :])
                    wpd.append(t_)

                def r3t(t_):   # [64, 2*N] tile -> [64, 2, N] AP
                    return t_[:].rearrange("p (two n) -> p two n", two=2)

                # ---- persistent attention operands ----
                qt = [pers.tile([P, TQC], f8, tag=f"qt{j}", name=f"qt{j}")
                      for j in range(NJ)]
                kt = [pers.tile([P, T], f8, tag=f"kt{j}", name=f"kt{j}")
                      for j in range(NJ)]
                vaug = [pers.tile([P, H * VSLOT], f8, tag=f"va{t}", name=f"va{t}")
                        for t in range(NTK)]
                yt = [pers.tile([P, TQC], bf16, tag=f"yt{j}", name=f"yt{j}")
                      for j in range(NJ)]

                # ---- phase B1: V = x @ Wv + bv -> vaug (+ ones col) ----
                if phase_lim >= 1:
                    for tk in range(NTK):
                        ones_ap = vaug[tk][:].rearrange("p (h e) -> p h e",
                                                        e=VSLOT)
                        nc.gpsimd.memset(ones_ap[:, :, 64:65], 1.0)
                    for d2 in range(2):
                        for tk in range(NTK):
                            psv = psp.tile([P, 512], f32, tag="sc", bufs=4, name=f"psv{d2}_{tk}")
                            for i in range(NJ):
                                nc.tensor.matmul(
                                    psv[:],
                                    r3t(xtd[i])[:, :, tk * P:(tk + 1) * P],
                                    r3t(wvd[i])[:, :, d2 * 512:(d2 + 1) * 512],
                                    start=(i == 0), stop=(i == NJ - 1),
                                    perf_mode=DR)
                            dst = vaug[tk][:].rearrange("p (h e) -> p h e",
                                                        e=VSLOT)
                            nc.vector.scalar_tensor_tensor(
                                dst[:, 8 * d2:8 * d2 + 8, 0:64],
                                psv[:].rearrange("p (h d) -> p h d", d=D),
                                INV_SV,
                                bv_bc[:, d2 * 512:(d2 + 1) * 512].rearrange(
                                    "p (h d) -> p h d", d=D),
                                op0=ALU.mult, op1=ALU.add)

                # wq reuses the wv tile slots (V proj done by then)
                if phase_lim >= 2:
                    for i in range(NJ):
                        t_ = pers.tile([64, 2 * C], f8, tag=f"w8s_{i}",
                                       name=f"wq_{i}")
                        nc.sync.dma_start(
                            t_[:], w8[(0 * NJ + i) * 64:(0 * NJ + i + 1) * 64, :])
                        wqd.append(t_)

                # ---- phase B2: per c-chunk j: Q^T, K^T ----
                def qk_produce(j):
                    for blk in range(NQBLK):
                        psq = psp.tile([P, 512], f32, tag="sc", bufs=4,
                                       name=f"psq{j}_{blk}")
                        for i in range(NJ):
                            nc.tensor.matmul(
                                psq[:, 0:QB],
                                r3t(wqd[i])[:, :, j * P:(j + 1) * P],
                                r3t(xqd[i])[:, :, blk * QB:(blk + 1) * QB],
                                start=(i == 0), stop=(i == NJ - 1),
                                perf_mode=DR)
                        # qt = (psq + bq*) * mask*  (masked rows -> 0 scores)
                        nc.vector.scalar_tensor_tensor(
                            qt[j][:, blk * QB:(blk + 1) * QB], psq[:, 0:QB],
                            bq_t[:, j:j + 1],
                            mask_bc[:, blk * QB:(blk + 1) * QB],
                            op0=ALU.add, op1=ALU.mult)
                    for kb in range(T // 512):
                        psk = psp.tile([P, 512], f32, tag="sc", bufs=4,
                                       name=f"psk{j}_{kb}")
                        for i in range(NJ):
                            nc.tensor.matmul(
                                psk[:],
                                r3t(wkd[i])[:, :, j * P:(j + 1) * P],
                                r3t(xtd[i])[:, :, kb * 512:(kb + 1) * 512],
                                start=(i == 0), stop=(i == NJ - 1),
                                perf_mode=DR)
                        nc.vector.tensor_scalar(
                            kt[j][:, kb * 512:(kb + 1) * 512], psk[:],
                            INV_SK, bk_t[:, j:j + 1],
                            op0=ALU.mult, op1=ALU.add)

                def attn_chunk(j):
                    yaccs = {}
                    for hh in range(2):
                        for blk in range(NQBLK):
                            yaccs[(hh, blk)] = psp.tile(
                                [65, 512], f32, tag="yacc", bufs=4,
                                name=f"yacc{j}_{hh}_{blk}")
                    def issue_attv(tk, exs):
                        for hh in range(2):
                            h = 2 * j + hh
                            for blk in range(NQBLK):
                                nc.tensor.matmul(
                                    yaccs[(hh, blk)][0:65, 0:QB],
                                    vaug[tk][:, h * VSLOT:h * VSLOT + 65],
                                    exs[(hh, blk)][:],
                                    start=(tk == 0), stop=(tk == NTK - 1))

                    prev = None
                    for tk in range(NTK):
                        exs = {}
                        for hh in range(2):
                            pb = hh * 64
                            for blk in range(NQBLK):
                                pss = psp.tile([P, 512], f32, tag="sc", bufs=4,
                                               name=f"pss{j}_{hh}_{blk}")
                                nc.tensor.matmul(
                                    pss[:, 0:QB],
                                    kt[j][pb:pb + 64, tk * P:(tk + 1) * P],
                                    qt[j][pb:pb + 64,
                                          blk * QB:(blk + 1) * QB],
                                    start=True, stop=True,
                                    tile_position=(pb, 0))
                                ex = exp_pool.tile([P, QB], f8, tag="ex",
                                                   name=f"ex{j}_{hh}_{blk}")
                                nc.scalar.activation(ex[:], pss[:, 0:QB],
                                                     ACTF.Exp)
                                exs[(hh, blk)] = ex
                        # attV for the PREVIOUS key chunk: PE never waits on
                        # the Act engine's exp of the current chunk
                        if prev is not None:
                            issue_attv(tk - 1, prev)
                        prev = exs
                    issue_attv(NTK - 1, prev)
                    for hh in range(2):
                        sr = smp.tile([P, TQC], f32, tag="sr")
                        for blk in range(NQBLK):
                            nc.vector.reciprocal(
                                sr[64:65, blk * QB:(blk + 1) * QB],
                                yaccs[(hh, blk)][64:65, 0:QB])
                        srb = smp.tile([1, TQC], f32, tag="srb", bufs=1)
                        nc.sync.dma_start(srb[:], sr[64:65, :])
                        nc.gpsimd.partition_broadcast(sr[0:64, :], srb[:])
                        if hh == 0:
                            for blk in range(NQBLK):
                                nc.vector.tensor_tensor(
                                    yt[j][0:64, blk * QB:(blk + 1) * QB],
                                    yaccs[(hh, blk)][0:64, 0:QB],
                                    sr[0:64, blk * QB:(blk + 1) * QB],
                                    op=ALU.mult)
                        else:
                            yo = smp.tile([64, TQC], bf16, tag="yo")
                            for blk in range(NQBLK):
                                nc.vector.tensor_tensor(
                                    yo[:, blk * QB:(blk + 1) * QB],
                                    yaccs[(hh, blk)][0:64, 0:QB],
                                    sr[0:64, blk * QB:(blk + 1) * QB],
                                    op=ALU.mult)
                            nc.sync.dma_start(yt[j][64:128, :], yo[:])

                if phase_lim >= 2:
                    if phase_lim < 3:
                        for j in range(NJ):
                            qk_produce(j)
                    else:
                        for j in range(NJ):
                            qk_produce(j)
                            attn_chunk(j)

                # ---- phase D: out proj + residual + LayerNorm ----
                if phase_lim >= 4:
                    # z = mean(V) @ Wp + bp from the meanV column (slot 639,
                    # guaranteed masked -> its normalized y == mean(V))
                    zc = pers.tile([1, C], f32, tag="zc")
                    for half in range(2):
                        psz = psp.tile([P, 512], f32, tag="sc", bufs=4,
                                       name=f"psz{half}")
                        for j in range(NJ):
                            nc.tensor.matmul(
                                psz[0:1, :],
                                yt[j][:, TQC - 1:TQC],
                                wpd[j][:, half * 512:(half + 1) * 512],
                                start=(j == 0), stop=(j == NJ - 1))
                        nc.vector.tensor_tensor(
                            zc[:, half * 512:(half + 1) * 512], psz[0:1, :],
                            bp_bc[0:1, half * 512:(half + 1) * 512],
                            op=ALU.add)
                    zbc = pers.tile([P, C], f32, tag="zbc")
                    nc.gpsimd.partition_broadcast(zbc[:], zc[:])

                    for i in range(TQ // P):
                        xr = evp.tile([P, C], f32, tag="xr", bufs=2,
                                      name=f"xr{i}")
                        nc.sync.dma_start(xr[:], fx[i * P:(i + 1) * P, :])
                        hres = evp.tile([P, C], f32, tag="hres", bufs=2)
                        if i < NQB:
                            for half in range(2):
                                pso = psp.tile([P, 512], f32, tag="sc", bufs=4,
                                               name=f"pso{i}_{half}")
                                for j in range(NJ):
                                    nc.tensor.matmul(
                                        pso[:],
                                        yt[j][:, i * P:(i + 1) * P],
                                        wpd[j][:, half * 512:(half + 1) * 512],
                                        start=(j == 0), stop=(j == NJ - 1))
                                nc.vector.tensor_tensor(
                                    hres[:, half * 512:(half + 1) * 512],
                                    pso[:],
                                    bp_bc[:, half * 512:(half + 1) * 512],
                                    op=ALU.add)
                            nc.vector.tensor_tensor(hres[:], hres[:], xr[:],
                                                    op=ALU.add)
                        else:
                            nc.vector.tensor_tensor(hres[:], zbc[:], xr[:],
                                                    op=ALU.add)
                        stat = smp.tile([P, 8], f32, tag="stat")
                        nc.vector.reduce_sum(stat[:, 0:1], hres[:], axis=AX.X)
                        sq = evp.tile([P, C], bf16, tag="sq", bufs=1)
                        nc.scalar.activation(sq[:], hres[:], ACTF.Square,
                                             accum_out=stat[:, 1:2])
                        nc.vector.tensor_scalar(stat[:, 2:3], stat[:, 0:1],
                                                1.0 / C, None, op0=ALU.mult)
                        nc.vector.tensor_scalar(stat[:, 3:4], stat[:, 1:2],
                                                1.0 / C, None, op0=ALU.mult)
                        nc.vector.tensor_tensor(stat[:, 4:5], stat[:, 2:3],
                                                stat[:, 2:3], op=ALU.mult)
                        nc.vector.tensor_tensor(stat[:, 5:6], stat[:, 3:4],
                                                stat[:, 4:5], op=ALU.subtract)
                        nc.scalar.activation(stat[:, 6:7], stat[:, 5:6],
                                             ACTF.Sqrt, bias=eps_t[:])
                        nc.vector.reciprocal(stat[:, 7:8], stat[:, 6:7])
                        nc.vector.tensor_scalar(hres[:], hres[:], stat[:, 2:3],
                                                stat[:, 7:8],
                                                op0=ALU.subtract, op1=ALU.mult)
                        if affine:
                            nc.vector.tensor_tensor(hres[:], hres[:],
                                                    lng_bc[:], op=ALU.mult)
                            nc.vector.tensor_tensor(hres[:], hres[:],
                                                    lnb_bc[:], op=ALU.add)
                        nc.sync.dma_start(outd[i * P:(i + 1) * P, :], hres[:])

            for _rep in range(repeat):
                _iter_body()

    nc.compile()
    return nc


# ---- host-side scales (set at import; recomputed per input in _make_in_maps) ----
INV_SV = 1.0
INV_SK = 1.0


_CACHE = {}


def _get_nc(affine: bool):
    if affine not in _CACHE:
        _CACHE[affine] = build(affine)
    return _CACHE[affine]


def _f8_layout(wT: np.ndarray, ncols: int, f8np) -> np.ndarray:
    """[C, ncols] f32 -> [NJ*64, 2*ncols] fp8 DoubleRow layout."""
    return (wT.reshape(NJ, 2, 64, ncols).transpose(0, 2, 1, 3)
            .reshape(NJ * 64, 2 * ncols).astype(f8np))


def _make_in_maps(x, Wq, bq, Wk, bk, Wv, bv, Wp, bp, ln_g, ln_b, mask,
                  affine: bool):
    global INV_SV, INV_SK
    f8np = mybir.dt.np(f8)
    bfnp = mybir.dt.np(bf16)
    x = np.asarray(x, np.float32)
    mask = np.asarray(mask)
    Wq = np.asarray(Wq, np.float32)
    Wk = np.asarray(Wk, np.float32)
    Wv = np.asarray(Wv, np.float32)
    Wp = np.asarray(Wp, np.float32)
    sc = np.float32(1.0 / np.sqrt(D))

    # fp8 scales: normalize each W to absmax ~224
    aQ = np.float32(224.0 / np.abs(Wq * sc).max())
    sK = np.float32(224.0 / np.abs(Wk).max())
    sV = np.float32(224.0 / np.abs(Wv).max())
    # these must match what build() baked in; build reads them at trace time,
    # so they are fixed constants of the compiled kernel -> bake via globals
    # before _get_nc() is called (see run()).
    INV_SV = float(1.0 / sV)
    INV_SK = float(1.0 / sK)

    w8_h = np.concatenate([
        _f8_layout(Wq * (sc * aQ), C, f8np),
        _f8_layout(Wk * sK, C, f8np),
        _f8_layout(Wv * sV, C, f8np)], axis=0)
    wpb_h = Wp.astype(bfnp)

    extra = np.stack([
        np.asarray(bq, np.float32) * sc * aQ,
        np.asarray(bk, np.float32),          # bk added after 1/sK rescale
        np.asarray(bv, np.float32),
        np.asarray(bp, np.float32),
        np.asarray(ln_g, np.float32), np.asarray(ln_b, np.float32),
        np.zeros(C, np.float32)], axis=0)

    in_maps = []
    perms = []
    for b in range(B):
        xb = x[b]
        xt8_h = _f8_layout(xb.T, T, f8np)
        mb = np.asarray(mask[b]) != 0
        U = np.nonzero(mb)[0]
        M = np.nonzero(~mb)[0]
        UA, UB = U[0::2], U[1::2]
        assert len(UA) <= TQC - 1 and len(UB) <= TQC - 1, (
            f"unmasked rows per core ({len(UA)}, {len(UB)}) exceed compact "
            f"capacity {TQC - 1}; mask too dense for this kernel build")
        nMA = TQ - len(UA)
        MA, MB = M[:nMA], M[nMA:]
        for half, (Uh, Mh) in enumerate(((UA, MA), (UB, MB))):
            perm = np.concatenate([Uh, Mh])
            perms.append(perm)
            fx_h = np.empty((TQ + 7, C), np.float32)
            fx_h[0:TQ] = xb[perm]
            fx_h[TQ:] = extra
            fx_h[TQ + 6, :] = 0.0
            fx_h[TQ + 6, 0:TQC] = mb[perm[:TQC]].astype(np.float32) / aQ
            xq8_h = _f8_layout(xb[perm[:TQC]].T, TQC, f8np)
            in_maps.append({
                "xt8": xt8_h, "xq8": xq8_h, "w8": w8_h, "wpb": wpb_h,
                "fx": fx_h,
            })
    return in_maps, perms


def run(inputs: dict, trace: bool = False):
    ln_g = np.asarray(inputs["ln_g"], np.float32)
    ln_b = np.asarray(inputs["ln_b"], np.float32)
    affine = not (np.all(ln_g == 1.0) and np.all(ln_b == 0.0))
    in_maps, perms = _make_in_maps(**inputs, affine=affine)
    nc = _get_nc(affine)
    res = None
    for attempt in range(3):
        try:
            res = run_bass_kernel_spmd(nc, in_maps, list(range(N_CORES)),
                                       trace=trace)
            break
        except Exception:
            if attempt == 2:
                raise
            import time as _time
            _time.sleep(2.0)
    out = np.empty((B, T, C), np.float32)
    for c in range(N_CORES):
        b = c // 2
        out[b, perms[c]] = res.results[c]["out"]
    return out, res


def kernel(**inputs) -> np.ndarray:
    out, _ = run(inputs, trace=False)
    return out
